# revision 42
# baseline (speedup 1.0000x reference)
"""TRN2 Bass kernel for nn_AttentionBlock (GroupNorm32 + 8-head attention + proj + residual).

Sharding: data-parallel over batch — batch=8, one batch element per NeuronCore, no
collectives.

Schedule (v1): the softmax exp stream on ScalarE (64 x (128,1024) activations,
~73us) is the critical path; everything else is arranged so ACT does *only* exp
from ~12us onward:
  - GroupNorm stats split across engines per 128-channel chunk as its DMA lands:
    Square+accum on ACT, row-sum on DVE, group-combine via tiny PE matmuls,
    rsqrt as exp(-0.5*ln v) (tiny, ACT), xhat affine on GpSimd.
  - qkv for head-pair 0 (q,k) accumulates per-chunk in PSUM so scores start
    ~1us after the last chunk's xhat.
  - Scores for a head pair run packed in the two 64-row PE groups; exp on ACT;
    attention matmul uses vT with an appended ones-column so the softmax
    denominator falls out of the same accumulation (lagged 2 blocks for slack).
  - Softmax division: PSUM accumulators staged to SBUF bf16, denominator rows
    DMA-folded to (128,16), reciprocal on DVE (RECIPROCAL_APPROX_FAST custom
    op), unfolded by DMA, broadcast on GpSimd (final pair: tiny PE outer
    products into spare PSUM banks) and multiplied on DVE in bf16.
  - vT and the q/k for pairs 1-3 are emitted inside the stream as PE filler;
    proj runs k-outer waves across all 8 PSUM banks overlapping the final
    division.

Numerics: all matmuls bf16 with fp32 PSUM accumulation; softmax denominators
pass through bf16 once (staging + reciprocal), everything else fp32.

Self-contained: hardcodes shapes from the problem spec (x (8,512,32,32) f32 etc).
"""
import numpy as np
import ml_dtypes

B, CH, HH, WW = 8, 512, 32, 32
L = HH * WW                  # 1024
HEADS = 8
GROUPS = 32
EPS = 1e-5
DH = CH // HEADS             # 64
KC = CH // 128               # 4 c-chunks
OC3 = 3 * CH // 128          # 12 qkv o-chunks
SC = L // 128                # 8 s/l-chunks
TC = L // 512                # 2 t-chunks
GN_N = (CH // GROUPS) * L    # elements per group = 16384
N_WARM = 12
ATTN_LAG = 2
SCOPES = False

_cache = {}


def _build(has_qkv_bias, has_proj_bias, debug=False):
    import concourse.bass as bass
    import concourse.tile as tile
    from concourse import bacc, mybir
    import bass_rust as _bass_rust
    from concourse.hw_specs import get_activation_tables
    from concourse.dve_ops import RECIPROCAL_APPROX_FAST, RECIP_APPROX_FAST_CONSTS

    F32 = mybir.dt.float32
    BF16 = mybir.dt.bfloat16
    AF = mybir.ActivationFunctionType
    OP = mybir.AluOpType
    AX = mybir.AxisListType

    class _Bacc(bacc.Bacc):
        # Pin Exp/Ln to the combined `natural_log_exp_and_others` table set so
        # alternating Ln/Exp activations don't thrash ACT_TABLE_LOADs (~2.7us
        # each). Same algorithm as Bacc.insert_act_table_loads, with Exp/Ln
        # stripped from every other set so the chooser can't pick them.
        def insert_act_table_loads(self):
            has_activation = any(
                isinstance(i, mybir.InstActivation)
                for b in self.main_func.blocks
                for i in b.instructions
            )
            if not has_activation:
                return
            combo = {AF.Exp, AF.Ln}
            tables = []
            for name, fns in get_activation_tables(self.m.arch).items():
                if name != "natural_log_exp_and_others":
                    fns = {f for f in fns if f not in combo}
                tables.append((name, fns))
            _bass_rust.insert_act_table_loads(self, tables)

    nc = _Bacc("TRN2", target_bir_lowering=False, debug=False, num_devices=8)

    x_d = nc.dram_tensor("x", [CH, L], F32, kind="ExternalInput").ap()
    qw_d = nc.dram_tensor("qkv_wt", [CH, 3 * CH], BF16, kind="ExternalInput").ap()
    pw_d = nc.dram_tensor("proj_wt", [CH, CH], BF16, kind="ExternalInput").ap()
    gmask_d = nc.dram_tensor("gmask", [128, 8], F32, kind="ExternalInput").ap()
    gmaskT_d = nc.dram_tensor("gmask_t", [8, 128], F32, kind="ExternalInput").ap()
    if has_qkv_bias:
        qkb_d = nc.dram_tensor("qk_bias", [128, 8], F32, kind="ExternalInput").ap()
        vb_d = nc.dram_tensor("v_bias", [128, KC], F32, kind="ExternalInput").ap()
    if has_proj_bias:
        pb_d = nc.dram_tensor("p_bias", [128, KC], F32, kind="ExternalInput").ap()
    out_d = nc.dram_tensor("out", [CH, L], F32, kind="ExternalOutput").ap()
    if debug:
        dbg = {
            "d_xhat": nc.dram_tensor("d_xhat", [128, KC * L], F32, kind="ExternalOutput").ap(),
            "d_qk": nc.dram_tensor("d_qk", [128, 8 * L], F32, kind="ExternalOutput").ap(),
            "d_vt": nc.dram_tensor("d_vt", [128, SC * HEADS * 65], F32, kind="ExternalOutput").ap(),
            "d_asb": nc.dram_tensor("d_asb", [128, KC * L], F32, kind="ExternalOutput").ap(),
            "d_ew0": nc.dram_tensor("d_ew0", [128, L], F32, kind="ExternalOutput").ap(),
        }

    with tile.TileContext(nc) as tc:
        import contextlib
        ctx = contextlib.ExitStack()
        pers = ctx.enter_context(tc.tile_pool(name="pers", bufs=1))
        scr = ctx.enter_context(tc.tile_pool(name="scr", bufs=2))
        ewp = ctx.enter_context(tc.tile_pool(name="ewp", bufs=10))
        dvp = ctx.enter_context(tc.tile_pool(name="dvp", bufs=2))
        asg = ctx.enter_context(tc.tile_pool(name="asg", bufs=8))
        outp = ctx.enter_context(tc.tile_pool(name="outp", bufs=3))

        import contextlib as _ctxlib
        def _scope(name):
            return tc.spectator_scope(name) if SCOPES else _ctxlib.nullcontext()

        # ---- load inputs, in consumption-priority order (DMA issue rate and
        # bandwidth are the startup critical path). x chunks 0-1 go out on the
        # GpSimd (SWDGE) queue, whose preamble ends ~3us before Sync's; Sync
        # takes the masks, x chunks 2-3, then pair-0's q/k weight slices, then
        # the rest of qkv_w, and proj_w (not needed until ~100us) last ----
        xs = pers.tile([128, KC * L], F32, tag="xs")
        qw = pers.tile([128, KC * 3 * CH], BF16, tag="qw")
        for k in range(KC):
            nc.sync.dma_start(xs[:, k * L:(k + 1) * L], x_d[128 * k:128 * (k + 1), :])
        gmask = pers.tile([128, 8], F32, tag="gmask")
        nc.sync.dma_start(gmask[:], gmask_d[:])
        gmaskT = pers.tile([8, 128], F32, tag="gmask_t")
        nc.sync.dma_start(gmaskT[:], gmaskT_d[:])
        for k in range(KC):
            for j in (0, 4):
                nc.sync.dma_start(
                    qw[:, k * 3 * CH + 128 * j:k * 3 * CH + 128 * (j + 1)],
                    qw_d[128 * k:128 * (k + 1), 128 * j:128 * (j + 1)])
        if has_qkv_bias:
            qkb = pers.tile([128, 8], F32, tag="qkb")
            nc.sync.dma_start(qkb[:], qkb_d[:])
            vb = pers.tile([128, KC], F32, tag="vb")
            nc.sync.dma_start(vb[:], vb_d[:])
        if has_proj_bias:
            pb = pers.tile([128, KC], F32, tag="pb")
            nc.sync.dma_start(pb[:], pb_d[:])
        ident_d = nc.dram_tensor("ident", [128, 128], BF16, kind="ExternalInput").ap()
        ident = pers.tile([128, 128], BF16, tag="ident")
        for k in range(KC):
            for c0, c1 in ((128, 512), (640, 1024), (1024, 1536)):
                nc.sync.dma_start(qw[:, k * 3 * CH + c0:k * 3 * CH + c1],
                                  qw_d[128 * k:128 * (k + 1), c0:c1])
        nc.sync.dma_start(ident[:], ident_d[:])
        pw = pers.tile([128, KC * CH], BF16, tag="pw")
        for k in range(KC):
            nc.sync.dma_start(pw[:, k * CH:(k + 1) * CH], pw_d[128 * k:128 * (k + 1), :])

        # ---- PE warmup: keep HAM at K=8/8 through the stats/DMA startup
        # chain (the pool stays open through stats so filler warm matmuls can
        # bridge the xhat-wait gaps) ----
        warm_pool = tc.tile_pool(name="psW", bufs=1, space="PSUM")
        psW = warm_pool.__enter__()
        wsrc = scr.tile([128, 640], BF16, tag="wsrc")
        nc.gpsimd.memset(wsrc[:], 0.0)
        wps = psW.tile([128, 512], F32, tag="warm")
        for _ in range(N_WARM):
            nc.tensor.matmul(wps[:], wsrc[:, 0:128], wsrc[:, 128:640],
                             start=True, stop=True)

        epsb = pers.tile([8, 1], F32, tag="epsb")
        nc.gpsimd.memset(epsb[:], EPS)
        ones64 = pers.tile([65, 64], BF16, tag="ones64")
        nc.gpsimd.memset(ones64[:], 1.0)
        # trigger the (single) ACT table load off the critical path
        tldt = pers.tile([8, 1], F32, tag="tldt")
        nc.scalar.activation(tldt[:], epsb[:], AF.Exp)

        # vT ones-columns memset early (GpSimd, idle here)
        vt = pers.tile([128, SC * (HEADS * 65)], BF16, tag="vt")
        for lc in range(SC):
            v3 = vt[:, lc * 520:(lc + 1) * 520].rearrange("p (h c) -> p h c", c=65)
            nc.gpsimd.memset(v3[:, :, 64:65], 1.0)

        # ---- GroupNorm statistics + xhat, per chunk, split across engines;
        # pair-0 q/k accumulates per chunk so scores can start right away ----
        qk = pers.tile([128, 8 * L], BF16, tag="qk")   # o-chunk j: cols j*L..; j=0-3 q, 4-7 k
        xhat = pers.tile([128, KC * L], BF16, tag="xhat")
        stat = pers.tile([128, 8], F32, tag="stat")  # cols 2k: sum(x), 2k+1: sum(x^2)
        bc = pers.tile([128, 2 * KC], F32, tag="bc")  # cols 2k mean, 2k+1 rstd

        psMisc = tc.tile_pool(name="psM", bufs=1, space="PSUM")
        psm = psMisc.__enter__()
        psQ0 = tc.tile_pool(name="psQ0", bufs=1, space="PSUM")
        psq0 = psQ0.__enter__()
        q0t = {}
        for j in (0, 4):
            for t in range(TC):
                q0t[(j, t)] = psq0.tile([128, 512], F32, tag=f"q0_{j}_{t}",
                                        name=f"q0_{j}_{t}")

        with _scope("stats"):
            for k in range(KC):
                xk = xs[:, k * L:(k + 1) * L]
                sq = scr.tile([128, L], F32, tag="sq")
                nc.scalar.activation(sq[:], xk, AF.Square,
                                     accum_out=stat[:, 2 * k + 1:2 * k + 2])
                nc.vector.reduce_sum(stat[:, 2 * k:2 * k + 1], xk, AX.X)
                gst_ps = psm.tile([8, 2], F32, tag="gs")
                nc.tensor.matmul(gst_ps[:], gmask[:], stat[:, 2 * k:2 * k + 2],
                                 start=True, stop=True)
                s2k = pers.tile([8, 2], F32, tag=f"s2k{k}")   # col 0 mean, col 1 rstd
                vk = pers.tile([8, 2], F32, tag=f"vk{k}")     # col 0 var, col 1 scratch
                nc.vector.tensor_scalar_mul(s2k[:], gst_ps[:], 1.0 / GN_N)  # mean, E[x^2]
                nc.vector.tensor_mul(vk[:, 1:2], s2k[:, 0:1], s2k[:, 0:1])  # mean^2
                nc.vector.tensor_sub(vk[:, 0:1], s2k[:, 1:2], vk[:, 1:2])   # var
                nc.scalar.activation(vk[:, 1:2], vk[:, 0:1], AF.Ln, bias=epsb[:])
                nc.scalar.activation(s2k[:, 1:2], vk[:, 1:2], AF.Exp, scale=-0.5)
                bc_ps = psm.tile([128, 2], F32, tag="bc")
                nc.tensor.matmul(bc_ps[:], gmaskT[:], s2k[:], start=True, stop=True)
                nc.vector.tensor_copy(bc[:, 2 * k:2 * k + 2], bc_ps[:])
                nmr = pers.tile([128, 1], F32, tag=f"nmr{k}")   # -mean*rstd
                nc.vector.tensor_scalar(
                    out=nmr[:], in0=bc[:, 2 * k:2 * k + 1],
                    scalar1=bc[:, 2 * k + 1:2 * k + 2], scalar2=-1.0,
                    op0=OP.mult, op1=OP.mult)
                # chunk 3 is on the critical chain to the first exp: its xhat
                # runs on DVE (2x tensor_scalar), chunks 0-2 on idle GpSimd
                xh_eng = nc.vector if k == KC - 1 else nc.gpsimd
                xh_eng.tensor_scalar(
                    out=xhat[:, k * L:(k + 1) * L], in0=xk,
                    scalar1=bc[:, 2 * k + 1:2 * k + 2], scalar2=nmr[:],
                    op0=OP.mult, op1=OP.add)
                # pair-0 q/k partials for this chunk
                for j in (0, 4):
                    for t in range(TC):
                        nc.tensor.matmul(
                            q0t[(j, t)][:],
                            qw[:, k * 3 * CH + 128 * j:k * 3 * CH + 128 * (j + 1)],
                            xhat[:, k * L + 512 * t:k * L + 512 * (t + 1)],
                            start=(k == 0), stop=(k == KC - 1))



        # pair-0 q/k PSUM -> SBUF: q on DVE, k on ACT (both idle here)
        for t in range(TC):
            dst = qk[:, 0 * L + 512 * t:0 * L + 512 * (t + 1)]
            if has_qkv_bias:
                nc.vector.tensor_scalar_add(dst, q0t[(0, t)][:], qkb[:, 0:1])
            else:
                nc.vector.tensor_copy(dst, q0t[(0, t)][:])
        for t in range(TC):
            dst = qk[:, 4 * L + 512 * t:4 * L + 512 * (t + 1)]
            if has_qkv_bias:
                nc.vector.tensor_scalar_add(dst, q0t[(4, t)][:], qkb[:, 4:5])
            elif t == 0:
                # scores(0,0) only needs k columns 0-127: copy those first so
                # the first score matmul isn't gated on the full k copy
                nc.scalar.activation(dst[:, 0:128], q0t[(4, 0)][:, 0:128],
                                     AF.Identity)
                nc.scalar.activation(dst[:, 128:512], q0t[(4, 0)][:, 128:512],
                                     AF.Identity)
            else:
                nc.scalar.activation(dst, q0t[(4, t)][:], AF.Identity)
        psQ0.__exit__(None, None, None)
        psMisc.__exit__(None, None, None)
        warm_pool.__exit__(None, None, None)

        if debug:
            def dump_bf16(dram_ap, sb_ap, width):
                for off in range(0, width, 512):
                    w = min(512, width - off)
                    stg = outp.tile([128, 512], F32, tag="dstg")
                    nc.vector.tensor_copy(stg[:sb_ap.shape[0], :w],
                                          sb_ap[:, off:off + w])
                    nc.sync.dma_start(dram_ap[:sb_ap.shape[0], off:off + w],
                                      stg[:sb_ap.shape[0], :w])
            dump_bf16(dbg["d_xhat"], xhat[:], KC * L)

        # ---- attention stream ----
        a_sb = pers.tile([128, KC * L], BF16, tag="a_sb")
        xsb = pers.tile([128, 2 * L], BF16, tag="xsb")
        attn_acc = tc.tile_pool(name="psA", bufs=4, space="PSUM")
        psA = attn_acc.__enter__()
        attn_psum = tc.tile_pool(name="psS", bufs=2, space="PSUM")
        psS = attn_psum.__enter__()

        # filler: vt chunk computation (into a psS generation)
        def make_vt(lc):
            def f():
                ps = psS.tile([128, 512], F32, tag="ps")
                for k in range(KC):
                    nc.tensor.matmul(
                        ps[:], xhat[:, k * L + 128 * lc:k * L + 128 * (lc + 1)],
                        qw[:, k * 3 * CH + 2 * CH:k * 3 * CH + 3 * CH],
                        start=(k == 0), stop=(k == KC - 1))
                v3 = vt[:, lc * 520:(lc + 1) * 520].rearrange("p (h c) -> p h c", c=65)
                src = ps[:].rearrange("p (h c) -> p h c", c=64)
                nc.vector.tensor_copy(v3[:, :, 0:64], src)
            return f

        # filler: deferred q/k o-chunk, one t-half (4 matmuls + copy) per item
        # so no single filler exceeds ~1us of PE time
        def make_qk(j, t):
            def f():
                ps = psS.tile([128, 512], F32, tag="ps", name=f"qkf{j}{t}")
                for k in range(KC):
                    nc.tensor.matmul(
                        ps[:],
                        qw[:, k * 3 * CH + 128 * j:k * 3 * CH + 128 * (j + 1)],
                        xhat[:, k * L + 512 * t:k * L + 512 * (t + 1)],
                        start=(k == 0), stop=(k == KC - 1))
                dst = qk[:, j * L + 512 * t:j * L + 512 * (t + 1)]
                if has_qkv_bias:
                    nc.vector.tensor_scalar_add(dst, ps[:], qkb[:, j:j + 1])
                else:
                    nc.vector.tensor_copy(dst, ps[:])
            return f

        # block index (m*8+sc) -> list of filler thunks. Nothing before the
        # stream: the first-exp chain (xhat3 -> qkv ch3 -> copies -> scores)
        # must own the PE at startup.
        filler_at = {}
        filler_at[0] = [make_vt(0)]
        filler_at[1] = [make_vt(1)]
        filler_at[2] = [make_vt(2)]
        filler_at[3] = [make_vt(3), make_qk(1, 0)]
        filler_at[4] = [make_qk(1, 1)]
        filler_at[5] = [make_vt(4), make_qk(5, 0)]
        filler_at[6] = [make_vt(5), make_qk(5, 1)]
        filler_at[7] = [make_vt(6)]
        filler_at[8] = [make_vt(7)]
        filler_at[9] = [make_qk(2, 0)]
        filler_at[10] = [make_qk(2, 1)]
        filler_at[11] = [make_qk(6, 0)]
        filler_at[12] = [make_qk(6, 1)]
        filler_at[16] = [make_qk(3, 0)]
        filler_at[17] = [make_qk(3, 1)]
        filler_at[18] = [make_qk(7, 0)]
        filler_at[19] = [make_qk(7, 1)]

        rcp_consts = dict(s0=RECIP_APPROX_FAST_CONSTS["s0"],
                          s1=RECIP_APPROX_FAST_CONSTS["s1"],
                          imm2=RECIP_APPROX_FAST_CONSTS["imm2"])

        def div_recip(stgs, dma_eng=None):
            # Fold the four 512-wide ones-row sums into (128,16) via tiny
            # SBUF->SBUF DMAs (DMA engines are idle here) so the DVE
            # reciprocal is one cheap op, then unfold back to a partition-0
            # row for the broadcast.
            de = dma_eng if dma_eng is not None else nc.sync
            den128 = dvp.tile([128, 16], BF16, tag="d128")
            for i, (sg, e, t, mm_) in enumerate(stgs):
                de.dma_start(den128[:, 4 * i:4 * (i + 1)], sg[64:65, :])
            r128 = dvp.tile([128, 16], BF16, tag="r128")
            nc.vector._custom_dve(RECIPROCAL_APPROX_FAST, out=r128[:],
                                  in0=den128[:], **rcp_consts)
            rden = dvp.tile([1, 4 * 512], BF16, tag="rden")
            for i in range(4):
                nc.sync.dma_start(rden[0:1, 512 * i:512 * (i + 1)],
                                  r128[:, 4 * i:4 * (i + 1)])
            return rden

        def div_mul(rden, i, sg, e, t, mm_):
            bsb = dvp.tile([64, 512], BF16, tag="bsb")
            nc.gpsimd.partition_broadcast(bsb[:], rden[0:1, 512 * i:512 * (i + 1)])
            dst = a_sb[64 * e:64 * (e + 1),
                       mm_ * L + 512 * t:mm_ * L + 512 * (t + 1)]
            nc.vector.tensor_mul(dst, sg[0:64, :], bsb[:])
            if has_qkv_bias:
                nc.vector.tensor_scalar_add(
                    dst, dst, vb[64 * e:64 * (e + 1), mm_:mm_ + 1])

        def division_steps(stgs):
            # generator: one cheap step per scheduling slot
            rden = div_recip(stgs)
            yield
            for i, (sg, e, t, mm_) in enumerate(stgs):
                div_mul(rden, i, sg, e, t, mm_)
                if i % 2 == 1:
                    yield

        # ---- flat 32-block stream: block b = (m, sc). Attention matmuls lag
        # the score/exp stream by a global block count (3, tapering to 1 at
        # the end), so each pair's tail attn+staging spreads into the next
        # pair's blocks instead of lumping at pair boundaries. ----
        ps_a_of = {}
        ew_tiles = {}
        state = {"pending_div": None, "final_stgs": None, "final_rden4": None,
                 "dslice": None}
        fin_et = [(0, 0), (1, 0), (0, 1), (1, 1)]

        def q_ap(m, e, t):
            return qk[64 * e:64 * (e + 1), m * L + 512 * t:m * L + 512 * (t + 1)]

        def k_ap(m, e, sc):
            return qk[64 * e:64 * (e + 1),
                      (4 + m) * L + 128 * sc:(4 + m) * L + 128 * (sc + 1)]

        def attn_emit(m, sc):
            if m not in ps_a_of:
                pa = [[None, None], [None, None]]
                for e in range(2):
                    for t in range(TC):
                        pa[e][t] = psA.tile([65, 512], F32, tag="pa",
                                            name=f"pa_{m}_{e}_{t}")
                ps_a_of[m] = pa
            pa = ps_a_of[m]
            for e in range(2):
                ew = ew_tiles[(m, sc, e)]
                for t in range(TC):
                    nc.tensor.matmul(
                        pa[e][t][:],
                        vt[:, sc * 520 + (2 * m + e) * 65:
                           sc * 520 + (2 * m + e) * 65 + 65],
                        ew[:, 512 * t:512 * (t + 1)],
                        start=(sc == 0), stop=(sc == SC - 1))

        def stage_pair(m):
            # flush the previous pair's division before starting a new one
            if state["pending_div"] is not None:
                for _ in state["pending_div"]:
                    pass
            ps_a = ps_a_of[m]
            stgs = []
            if m == 3:
                # Final pair: no DMA fold round-trip (it would sit naked on
                # the critical path). The four denominator rows are copied
                # (split ACT/DVE) into one tile at base partitions 0/32/64
                # (cols 512:1024 of p64 for the 4th), one DVE reciprocal
                # covers all four, and the PE broadcasts from its rows during
                # proj. Value rows stage split ACT/DVE, t=0 entries first.
                den4 = dvp.tile([65, 1024], BF16, tag="den4")
                _dslice = lambda dd, i_: (dd[32 * i_:32 * i_ + 1, 0:512]
                                          if i_ < 3 else dd[64:65, 512:1024])
                for i_, (e, t) in enumerate(fin_et):
                    if i_ % 2 == 0:
                        nc.scalar.activation(_dslice(den4, i_),
                                             ps_a[e][t][64:65, :], AF.Identity)
                    else:
                        nc.vector.tensor_copy(_dslice(den4, i_),
                                              ps_a[e][t][64:65, :])
                rden4 = dvp.tile([65, 1024], BF16, tag="rden4")
                nc.vector._custom_dve(RECIPROCAL_APPROX_FAST, out=rden4[:],
                                      in0=den4[:], **rcp_consts)
                state["final_rden4"] = rden4
                state["dslice"] = _dslice
                for e, t in fin_et:
                    sg = asg.tile([64, 512], BF16, tag="astg")
                    if e == 0:
                        nc.scalar.activation(sg[:], ps_a[e][t][0:64, :],
                                             AF.Identity)
                    else:
                        nc.vector.tensor_copy(sg[:], ps_a[e][t][0:64, :])
                    stgs.append((sg, e, t, m))
                state["pending_div"] = None
                state["final_stgs"] = stgs
            else:
                for e in range(2):
                    for t in range(TC):
                        sg = asg.tile([65, 512], BF16, tag="astg")
                        nc.vector.tensor_copy(sg[:], ps_a[e][t][:])
                        stgs.append((sg, e, t, m))
                state["pending_div"] = division_steps(stgs)

        def lag_for(b):
            return 2 if b < 28 else 1

        pending_attn = []
        for b in range(4 * SC):
            m, sc = divmod(b, SC)
            ps_w = [None, None]
            for e in range(2):
                ps_w[e] = psS.tile([128, 1024], F32, tag="ps", name=f"ps_{b}_{e}")
            # packed score MM pairs (head 2m rows 0-63, head 2m+1 rows 64-127)
            for t in range(TC):
                for e in range(2):
                    nc.tensor.matmul(ps_w[e][:, 512 * t:512 * (t + 1)],
                                     k_ap(m, e, sc), q_ap(m, e, t),
                                     start=True, stop=True)
            for e in range(2):
                ew = ewp.tile([128, L], BF16, tag="ew")
                ew_tiles[(m, sc, e)] = ew
                nc.scalar.activation(ew[:], ps_w[e][:], AF.Exp)
            if debug and b == 0:
                dump_bf16(dbg["d_ew0"], ew_tiles[(0, 0, 0)][:], L)
            # pending division, one step per block to spread the load
            if state["pending_div"] is not None:
                next(state["pending_div"], None)
            # PE filler (vt chunks / deferred q,k) in the ACT-bound stream
            for f in filler_at.get(b, ()):
                f()
            if b == 16:
                # stage the residual input chunks 0-1 to bf16 for the
                # proj-tail identity matmuls (DVE has slack here)
                for ii in range(2):
                    for tt in range(TC):
                        nc.vector.tensor_copy(
                            xsb[:, ii * L + 512 * tt:ii * L + 512 * (tt + 1)],
                            xs[:, ii * L + 512 * tt:ii * L + 512 * (tt + 1)])
            # lagged attention drain
            pending_attn.append((m, sc))
            while len(pending_attn) > lag_for(b):
                am, asc = pending_attn.pop(0)
                attn_emit(am, asc)
                if asc == SC - 1:
                    stage_pair(am)
        while pending_attn:
            am, asc = pending_attn.pop(0)
            attn_emit(am, asc)
            if asc == SC - 1:
                stage_pair(am)
        final_stgs = state["final_stgs"]
        final_rden4 = state["final_rden4"]
        _dslice = state["dslice"]
        attn_psum.__exit__(None, None, None)

        if debug:
            dump_bf16(dbg["d_qk"], qk[:], 8 * L)
            dump_bf16(dbg["d_vt"], vt[:], SC * HEADS * 65)
            dump_bf16(dbg["d_asb"], a_sb[:], KC * L)

        # ---- proj + residual: k-outer waves, so the first 24 matmuls only
        # need a_sb chunks 0-2 and overlap the final division flush. The
        # final pair's reciprocal broadcasts run as tiny PE outer products
        # into the (now free) psA slots; psA stays open through proj.
        # For output chunks i=0,1 the residual is folded into PSUM via an
        # identity matmul on xs_bf16 so ScalarE can produce those outputs
        # (and issue their DMAs) in parallel with DVE doing i=2,3 ----
        with tc.tile_pool(name="psP", bufs=4, space="PSUM") as psP, \
             _scope("proj"):
            # final pair's reciprocal broadcasts (PE outer products into the
            # freed psA slots) + multiplies, t=0 entries first
            for i_, (sg, e, tt, mm_) in enumerate(final_stgs):
                pb_ps = psA.tile([64, 512], F32, tag="pa", name=f"pbc{i_}")
                bp = 32 * i_ if i_ < 3 else 64
                nc.tensor.matmul(pb_ps[:], ones64[bp:bp + 1, :],
                                 _dslice(final_rden4, i_), start=True, stop=True)
                dst = a_sb[64 * e:64 * (e + 1),
                           mm_ * L + 512 * tt:mm_ * L + 512 * (tt + 1)]
                nc.vector.tensor_mul(dst, sg[:], pb_ps[:])
                if has_qkv_bias:
                    nc.vector.tensor_scalar_add(
                        dst, dst, vb[64 * e:64 * (e + 1), mm_:mm_ + 1])
            # output tiles in slot order (i0t0,i1t0,i0t1,i1t1) then i2/i3:
            # group A (i=0,1, residual folded in via identity matmul on
            # xs_bf16) drains through ScalarE + its DMA queue; group B
            # (i=2,3) through DVE adds + the sync queue
            for i, t in ((0, 0), (1, 0), (0, 1), (1, 1),
                         (2, 0), (3, 0), (2, 1), (3, 1)):
                ps = psP.tile([128, 512], F32, tag="ps", name=f"pj{i}{t}")
                for k in range(KC):
                    nc.tensor.matmul(
                        ps[:],
                        pw[:, k * CH + 128 * i:k * CH + 128 * (i + 1)],
                        a_sb[:, k * L + 512 * t:k * L + 512 * (t + 1)],
                        start=(k == 0), stop=(k == KC - 1 and i >= 2))
                ot = outp.tile([128, 512], F32, tag="ot")
                if i < 2:
                    nc.tensor.matmul(
                        ps[:], ident[:],
                        xsb[:, i * L + 512 * t:i * L + 512 * (t + 1)],
                        start=False, stop=True)
                    nc.scalar.activation(ot[:], ps[:], AF.Identity)
                    if has_proj_bias:
                        nc.vector.tensor_scalar_add(ot[:], ot[:], pb[:, i:i + 1])
                    nc.scalar.dma_start(
                        out_d[128 * i:128 * (i + 1), 512 * t:512 * (t + 1)], ot[:])
                else:
                    nc.vector.tensor_add(
                        ot[:], xs[:, i * L + 512 * t:i * L + 512 * (t + 1)], ps[:])
                    if has_proj_bias:
                        nc.vector.tensor_scalar_add(ot[:], ot[:], pb[:, i:i + 1])
                    nc.sync.dma_start(
                        out_d[128 * i:128 * (i + 1), 512 * t:512 * (t + 1)], ot[:])
        attn_acc.__exit__(None, None, None)
        ctx.close()

    nc.compile()
    return nc


def _prep_inputs(x, norm_w, norm_b, qkv_w, qkv_b, proj_w, proj_b):
    scale = DH ** -0.25
    w_eff = (qkv_w.astype(np.float64) * norm_w.astype(np.float64)[None, :])
    b_eff = qkv_b.astype(np.float64) + w_eff @ norm_b.astype(np.float64)
    # reference splits qkv per head: row h*192 + {0:64 q, 64:128 k, 128:192 v}.
    # device layout wants [q_all_heads | k_all_heads | v_all_heads], head-major.
    perm = np.concatenate([
        np.concatenate([np.arange(h * 3 * DH + t * DH, h * 3 * DH + (t + 1) * DH)
                        for h in range(HEADS)])
        for t in range(3)])
    w_eff = w_eff[perm]
    b_eff = b_eff[perm]
    w_eff[:2 * CH] *= scale
    b_eff[:2 * CH] *= scale
    qkv_wt = np.ascontiguousarray(w_eff.T).astype(np.float32).astype(ml_dtypes.bfloat16)
    proj_wt = np.ascontiguousarray(proj_w.T).astype(ml_dtypes.bfloat16)

    p = np.arange(128)
    gmask = (p[:, None] // 16 == np.arange(8)[None, :]).astype(np.float32)
    gmask_t = np.ascontiguousarray(gmask.T)

    has_qkv_bias = bool(np.any(b_eff != 0.0))
    has_proj_bias = bool(np.any(proj_b != 0.0))
    common = {"qkv_wt": qkv_wt, "proj_wt": proj_wt, "gmask": gmask,
              "gmask_t": gmask_t,
              "ident": np.eye(128, dtype=np.float32).astype(ml_dtypes.bfloat16)}
    if has_qkv_bias:
        qk_part = b_eff[:2 * CH].astype(np.float32).reshape(8, 128).T
        v_part = b_eff[2 * CH:].astype(np.float32).reshape(KC, 128).T
        common["qk_bias"] = np.ascontiguousarray(qk_part)
        common["v_bias"] = np.ascontiguousarray(v_part)
    if has_proj_bias:
        common["p_bias"] = np.ascontiguousarray(
            proj_b.astype(np.float32).reshape(KC, 128).T)
    xf = np.ascontiguousarray(x.reshape(B, CH, L)).astype(np.float32)
    in_maps = [dict(common, x=np.ascontiguousarray(xf[i])) for i in range(B)]
    return in_maps, has_qkv_bias, has_proj_bias


def _get_nc(flags):
    if flags not in _cache:
        _cache[flags] = _build(*flags)
    return _cache[flags]


def _run(inputs, trace=False, tmpdir=None):
    import time
    from concourse.bass_utils import run_bass_kernel_spmd
    in_maps, hqb, hpb = _prep_inputs(**inputs)
    nc = _get_nc((hqb, hpb))
    kw = {}
    if trace:
        kw = dict(trace=True, tmpdir=tmpdir)
    last_err = None
    for attempt in range(3):
        # the very first execution on a freshly-attached device occasionally
        # fails with NRT_EXEC_UNIT_UNRECOVERABLE; a retry recovers it
        try:
            res = run_bass_kernel_spmd(nc, in_maps, list(range(B)), **kw)
            break
        except Exception as e:  # noqa: BLE001
            last_err = e
            time.sleep(5)
    else:
        raise last_err
    out = np.stack([res.results[i]["out"] for i in range(B)])
    return out.reshape(B, CH, HH, WW).astype(np.float32), res


def kernel(x, norm_w, norm_b, qkv_w, qkv_b, proj_w, proj_b):
    out, _ = _run(dict(x=x, norm_w=norm_w, norm_b=norm_b, qkv_w=qkv_w,
                       qkv_b=qkv_b, proj_w=proj_w, proj_b=proj_b))
    return out


# revision 44
# speedup vs baseline: 1.0060x; 1.0060x over previous
"""TRN2 Bass kernel for nn_AttentionBlock (GroupNorm32 + 8-head attention + proj + residual).

Sharding: data-parallel over batch — batch=8, one batch element per NeuronCore, no
collectives.

Schedule (v1): the softmax exp stream on ScalarE (64 x (128,1024) activations,
~73us) is the critical path; everything else is arranged so ACT does *only* exp
from ~12us onward:
  - GroupNorm stats split across engines per 128-channel chunk as its DMA lands:
    Square+accum on ACT, row-sum on DVE, group-combine via tiny PE matmuls,
    rsqrt as exp(-0.5*ln v) (tiny, ACT), xhat affine on GpSimd.
  - qkv for head-pair 0 (q,k) accumulates per-chunk in PSUM so scores start
    ~1us after the last chunk's xhat.
  - Scores for a head pair run packed in the two 64-row PE groups; exp on ACT;
    attention matmul uses vT with an appended ones-column so the softmax
    denominator falls out of the same accumulation (lagged 2 blocks for slack).
  - Softmax division: PSUM accumulators staged to SBUF bf16, denominator rows
    DMA-folded to (128,16), reciprocal on DVE (RECIPROCAL_APPROX_FAST custom
    op), unfolded by DMA, broadcast on GpSimd (final pair: tiny PE outer
    products into spare PSUM banks) and multiplied on DVE in bf16.
  - vT and the q/k for pairs 1-3 are emitted inside the stream as PE filler;
    proj runs k-outer waves across all 8 PSUM banks overlapping the final
    division.

Numerics: all matmuls bf16 with fp32 PSUM accumulation; softmax denominators
pass through bf16 once (staging + reciprocal), everything else fp32.

Self-contained: hardcodes shapes from the problem spec (x (8,512,32,32) f32 etc).
"""
import numpy as np
import ml_dtypes

B, CH, HH, WW = 8, 512, 32, 32
L = HH * WW                  # 1024
HEADS = 8
GROUPS = 32
EPS = 1e-5
DH = CH // HEADS             # 64
KC = CH // 128               # 4 c-chunks
OC3 = 3 * CH // 128          # 12 qkv o-chunks
SC = L // 128                # 8 s/l-chunks
TC = L // 512                # 2 t-chunks
GN_N = (CH // GROUPS) * L    # elements per group = 16384
N_WARM = 12
ATTN_LAG = 2
SCOPES = False

_cache = {}


def _build(has_qkv_bias, has_proj_bias, debug=False):
    import concourse.bass as bass
    import concourse.tile as tile
    from concourse import bacc, mybir
    import bass_rust as _bass_rust
    from concourse.hw_specs import get_activation_tables
    from concourse.dve_ops import RECIPROCAL_APPROX_FAST, RECIP_APPROX_FAST_CONSTS

    F32 = mybir.dt.float32
    BF16 = mybir.dt.bfloat16
    AF = mybir.ActivationFunctionType
    OP = mybir.AluOpType
    AX = mybir.AxisListType

    class _Bacc(bacc.Bacc):
        # Pin Exp/Ln to the combined `natural_log_exp_and_others` table set so
        # alternating Ln/Exp activations don't thrash ACT_TABLE_LOADs (~2.7us
        # each). Same algorithm as Bacc.insert_act_table_loads, with Exp/Ln
        # stripped from every other set so the chooser can't pick them.
        def insert_act_table_loads(self):
            has_activation = any(
                isinstance(i, mybir.InstActivation)
                for b in self.main_func.blocks
                for i in b.instructions
            )
            if not has_activation:
                return
            combo = {AF.Exp, AF.Ln}
            tables = []
            for name, fns in get_activation_tables(self.m.arch).items():
                if name != "natural_log_exp_and_others":
                    fns = {f for f in fns if f not in combo}
                tables.append((name, fns))
            _bass_rust.insert_act_table_loads(self, tables)

    nc = _Bacc("TRN2", target_bir_lowering=False, debug=False, num_devices=8)

    x_d = nc.dram_tensor("x", [CH, L], F32, kind="ExternalInput").ap()
    qw_d = nc.dram_tensor("qkv_wt", [CH, 3 * CH], BF16, kind="ExternalInput").ap()
    pw_d = nc.dram_tensor("proj_wt", [CH, CH], BF16, kind="ExternalInput").ap()
    gmask_d = nc.dram_tensor("gmask", [128, 8], F32, kind="ExternalInput").ap()
    gmaskT_d = nc.dram_tensor("gmask_t", [8, 128], F32, kind="ExternalInput").ap()
    if has_qkv_bias:
        qkb_d = nc.dram_tensor("qk_bias", [128, 8], F32, kind="ExternalInput").ap()
        vb_d = nc.dram_tensor("v_bias", [128, KC], F32, kind="ExternalInput").ap()
    if has_proj_bias:
        pb_d = nc.dram_tensor("p_bias", [128, KC], F32, kind="ExternalInput").ap()
    out_d = nc.dram_tensor("out", [CH, L], F32, kind="ExternalOutput").ap()
    if debug:
        dbg = {
            "d_xhat": nc.dram_tensor("d_xhat", [128, KC * L], F32, kind="ExternalOutput").ap(),
            "d_qk": nc.dram_tensor("d_qk", [128, 8 * L], F32, kind="ExternalOutput").ap(),
            "d_vt": nc.dram_tensor("d_vt", [128, SC * HEADS * 65], F32, kind="ExternalOutput").ap(),
            "d_asb": nc.dram_tensor("d_asb", [128, KC * L], F32, kind="ExternalOutput").ap(),
            "d_ew0": nc.dram_tensor("d_ew0", [128, L], F32, kind="ExternalOutput").ap(),
        }

    with tile.TileContext(nc) as tc:
        import contextlib
        ctx = contextlib.ExitStack()
        pers = ctx.enter_context(tc.tile_pool(name="pers", bufs=1))
        scr = ctx.enter_context(tc.tile_pool(name="scr", bufs=2))
        ewp = ctx.enter_context(tc.tile_pool(name="ewp", bufs=10))
        dvp = ctx.enter_context(tc.tile_pool(name="dvp", bufs=2))
        asg = ctx.enter_context(tc.tile_pool(name="asg", bufs=8))
        outp = ctx.enter_context(tc.tile_pool(name="outp", bufs=3))

        import contextlib as _ctxlib
        def _scope(name):
            return tc.spectator_scope(name) if SCOPES else _ctxlib.nullcontext()

        # ---- load inputs, in consumption-priority order (DMA issue rate and
        # bandwidth are the startup critical path). x chunks 0-1 go out on the
        # GpSimd (SWDGE) queue, whose preamble ends ~3us before Sync's; Sync
        # takes the masks, x chunks 2-3, then pair-0's q/k weight slices, then
        # the rest of qkv_w, and proj_w (not needed until ~100us) last ----
        xs = pers.tile([128, KC * L], F32, tag="xs")
        qw = pers.tile([128, KC * 3 * CH], BF16, tag="qw")
        for k in range(KC):
            nc.sync.dma_start(xs[:, k * L:(k + 1) * L], x_d[128 * k:128 * (k + 1), :])
        gmask = pers.tile([128, 8], F32, tag="gmask")
        nc.sync.dma_start(gmask[:], gmask_d[:])
        gmaskT = pers.tile([8, 128], F32, tag="gmask_t")
        nc.sync.dma_start(gmaskT[:], gmaskT_d[:])
        for k in range(KC):
            for j in (0, 4):
                nc.sync.dma_start(
                    qw[:, k * 3 * CH + 128 * j:k * 3 * CH + 128 * (j + 1)],
                    qw_d[128 * k:128 * (k + 1), 128 * j:128 * (j + 1)])
        if has_qkv_bias:
            qkb = pers.tile([128, 8], F32, tag="qkb")
            nc.sync.dma_start(qkb[:], qkb_d[:])
            vb = pers.tile([128, KC], F32, tag="vb")
            nc.sync.dma_start(vb[:], vb_d[:])
        if has_proj_bias:
            pb = pers.tile([128, KC], F32, tag="pb")
            nc.sync.dma_start(pb[:], pb_d[:])
        ident_d = nc.dram_tensor("ident", [128, 128], BF16, kind="ExternalInput").ap()
        ident = pers.tile([128, 128], BF16, tag="ident")
        for k in range(KC):
            for c0, c1 in ((128, 512), (640, 1024), (1024, 1536)):
                nc.sync.dma_start(qw[:, k * 3 * CH + c0:k * 3 * CH + c1],
                                  qw_d[128 * k:128 * (k + 1), c0:c1])
        nc.sync.dma_start(ident[:], ident_d[:])
        pw = pers.tile([128, KC * CH], BF16, tag="pw")
        for k in range(KC):
            nc.sync.dma_start(pw[:, k * CH:(k + 1) * CH], pw_d[128 * k:128 * (k + 1), :])

        # ---- PE warmup: keep HAM at K=8/8 through the stats/DMA startup
        # chain (the pool stays open through stats so filler warm matmuls can
        # bridge the xhat-wait gaps) ----
        warm_pool = tc.tile_pool(name="psW", bufs=1, space="PSUM")
        psW = warm_pool.__enter__()
        wsrc = scr.tile([128, 640], BF16, tag="wsrc")
        nc.gpsimd.memset(wsrc[:], 0.0)
        wps = psW.tile([128, 512], F32, tag="warm")
        for _ in range(N_WARM):
            nc.tensor.matmul(wps[:], wsrc[:, 0:128], wsrc[:, 128:640],
                             start=True, stop=True)

        epsb = pers.tile([8, 1], F32, tag="epsb")
        nc.gpsimd.memset(epsb[:], EPS)
        ones64 = pers.tile([65, 64], BF16, tag="ones64")
        nc.gpsimd.memset(ones64[:], 1.0)
        # trigger the (single) ACT table load off the critical path
        tldt = pers.tile([8, 1], F32, tag="tldt")
        nc.scalar.activation(tldt[:], epsb[:], AF.Exp)

        # vT ones-columns memset early (GpSimd, idle here)
        vt = pers.tile([128, SC * (HEADS * 65)], BF16, tag="vt")
        for lc in range(SC):
            v3 = vt[:, lc * 520:(lc + 1) * 520].rearrange("p (h c) -> p h c", c=65)
            nc.gpsimd.memset(v3[:, :, 64:65], 1.0)

        # ---- GroupNorm statistics + xhat, per chunk, split across engines;
        # pair-0 q/k accumulates per chunk so scores can start right away ----
        qk = pers.tile([128, 8 * L], BF16, tag="qk")   # o-chunk j: cols j*L..; j=0-3 q, 4-7 k
        xhat = pers.tile([128, KC * L], BF16, tag="xhat")
        stat = pers.tile([128, 8], F32, tag="stat")  # cols 2k: sum(x), 2k+1: sum(x^2)
        bc = pers.tile([128, 2 * KC], F32, tag="bc")  # cols 2k mean, 2k+1 rstd

        psMisc = tc.tile_pool(name="psM", bufs=1, space="PSUM")
        psm = psMisc.__enter__()
        psQ0 = tc.tile_pool(name="psQ0", bufs=1, space="PSUM")
        psq0 = psQ0.__enter__()
        q0t = {}
        for j in (0, 4):
            for t in range(TC):
                q0t[(j, t)] = psq0.tile([128, 512], F32, tag=f"q0_{j}_{t}",
                                        name=f"q0_{j}_{t}")

        with _scope("stats"):
            for k in range(KC):
                xk = xs[:, k * L:(k + 1) * L]
                sq = scr.tile([128, L], F32, tag="sq")
                nc.scalar.activation(sq[:], xk, AF.Square,
                                     accum_out=stat[:, 2 * k + 1:2 * k + 2])
                nc.vector.reduce_sum(stat[:, 2 * k:2 * k + 1], xk, AX.X)
                gst_ps = psm.tile([8, 2], F32, tag="gs")
                nc.tensor.matmul(gst_ps[:], gmask[:], stat[:, 2 * k:2 * k + 2],
                                 start=True, stop=True)
                s2k = pers.tile([8, 2], F32, tag=f"s2k{k}")   # col 0 mean, col 1 rstd
                vk = pers.tile([8, 2], F32, tag=f"vk{k}")     # col 0 var, col 1 scratch
                nc.vector.tensor_scalar_mul(s2k[:], gst_ps[:], 1.0 / GN_N)  # mean, E[x^2]
                nc.vector.tensor_mul(vk[:, 1:2], s2k[:, 0:1], s2k[:, 0:1])  # mean^2
                nc.vector.tensor_sub(vk[:, 0:1], s2k[:, 1:2], vk[:, 1:2])   # var
                nc.scalar.activation(vk[:, 1:2], vk[:, 0:1], AF.Ln, bias=epsb[:])
                nc.scalar.activation(s2k[:, 1:2], vk[:, 1:2], AF.Exp, scale=-0.5)
                bc_ps = psm.tile([128, 2], F32, tag="bc")
                nc.tensor.matmul(bc_ps[:], gmaskT[:], s2k[:], start=True, stop=True)
                nc.vector.tensor_copy(bc[:, 2 * k:2 * k + 2], bc_ps[:])
                nmr = pers.tile([128, 1], F32, tag=f"nmr{k}")   # -mean*rstd
                nc.vector.tensor_scalar(
                    out=nmr[:], in0=bc[:, 2 * k:2 * k + 1],
                    scalar1=bc[:, 2 * k + 1:2 * k + 2], scalar2=-1.0,
                    op0=OP.mult, op1=OP.mult)
                # chunk 3 is on the critical chain to the first exp: its xhat
                # runs on DVE (2x tensor_scalar), chunks 0-2 on idle GpSimd
                xh_eng = nc.vector if k == KC - 1 else nc.gpsimd
                xh_eng.tensor_scalar(
                    out=xhat[:, k * L:(k + 1) * L], in0=xk,
                    scalar1=bc[:, 2 * k + 1:2 * k + 2], scalar2=nmr[:],
                    op0=OP.mult, op1=OP.add)
                # pair-0 q/k partials for this chunk
                for j in (0, 4):
                    for t in range(TC):
                        nc.tensor.matmul(
                            q0t[(j, t)][:],
                            qw[:, k * 3 * CH + 128 * j:k * 3 * CH + 128 * (j + 1)],
                            xhat[:, k * L + 512 * t:k * L + 512 * (t + 1)],
                            start=(k == 0), stop=(k == KC - 1))



        # pair-0 q/k PSUM -> SBUF: q on DVE, k on ACT (both idle here)
        for t in range(TC):
            dst = qk[:, 0 * L + 512 * t:0 * L + 512 * (t + 1)]
            if has_qkv_bias:
                nc.vector.tensor_scalar_add(dst, q0t[(0, t)][:], qkb[:, 0:1])
            else:
                nc.vector.tensor_copy(dst, q0t[(0, t)][:])
        for t in range(TC):
            dst = qk[:, 4 * L + 512 * t:4 * L + 512 * (t + 1)]
            if has_qkv_bias:
                nc.vector.tensor_scalar_add(dst, q0t[(4, t)][:], qkb[:, 4:5])
            elif t == 0:
                # scores(0,0) only needs k columns 0-127: copy those first so
                # the first score matmul isn't gated on the full k copy
                nc.scalar.activation(dst[:, 0:128], q0t[(4, 0)][:, 0:128],
                                     AF.Identity)
                nc.scalar.activation(dst[:, 128:512], q0t[(4, 0)][:, 128:512],
                                     AF.Identity)
            else:
                nc.scalar.activation(dst, q0t[(4, t)][:], AF.Identity)
        psQ0.__exit__(None, None, None)
        psMisc.__exit__(None, None, None)
        warm_pool.__exit__(None, None, None)

        if debug:
            def dump_bf16(dram_ap, sb_ap, width):
                for off in range(0, width, 512):
                    w = min(512, width - off)
                    stg = outp.tile([128, 512], F32, tag="dstg")
                    nc.vector.tensor_copy(stg[:sb_ap.shape[0], :w],
                                          sb_ap[:, off:off + w])
                    nc.sync.dma_start(dram_ap[:sb_ap.shape[0], off:off + w],
                                      stg[:sb_ap.shape[0], :w])
            dump_bf16(dbg["d_xhat"], xhat[:], KC * L)

        # ---- attention stream ----
        a_sb = pers.tile([128, KC * L], BF16, tag="a_sb")
        xsb = pers.tile([128, 2 * L], BF16, tag="xsb")
        attn_acc = tc.tile_pool(name="psA", bufs=4, space="PSUM")
        psA = attn_acc.__enter__()
        attn_psum = tc.tile_pool(name="psS", bufs=2, space="PSUM")
        psS = attn_psum.__enter__()

        # filler: vt chunk computation (into a psS generation)
        def make_vt(lc):
            def f():
                ps = psS.tile([128, 512], F32, tag="ps")
                for k in range(KC):
                    nc.tensor.matmul(
                        ps[:], xhat[:, k * L + 128 * lc:k * L + 128 * (lc + 1)],
                        qw[:, k * 3 * CH + 2 * CH:k * 3 * CH + 3 * CH],
                        start=(k == 0), stop=(k == KC - 1))
                v3 = vt[:, lc * 520:(lc + 1) * 520].rearrange("p (h c) -> p h c", c=65)
                src = ps[:].rearrange("p (h c) -> p h c", c=64)
                nc.vector.tensor_copy(v3[:, :, 0:64], src)
            return f

        # filler: deferred q/k o-chunk, one t-half (4 matmuls + copy) per item
        # so no single filler exceeds ~1us of PE time
        def make_qk(j, t):
            def f():
                ps = psS.tile([128, 512], F32, tag="ps", name=f"qkf{j}{t}")
                for k in range(KC):
                    nc.tensor.matmul(
                        ps[:],
                        qw[:, k * 3 * CH + 128 * j:k * 3 * CH + 128 * (j + 1)],
                        xhat[:, k * L + 512 * t:k * L + 512 * (t + 1)],
                        start=(k == 0), stop=(k == KC - 1))
                dst = qk[:, j * L + 512 * t:j * L + 512 * (t + 1)]
                if has_qkv_bias:
                    nc.vector.tensor_scalar_add(dst, ps[:], qkb[:, j:j + 1])
                else:
                    nc.vector.tensor_copy(dst, ps[:])
            return f

        # block index (m*8+sc) -> list of filler thunks. Nothing before the
        # stream: the first-exp chain (xhat3 -> qkv ch3 -> copies -> scores)
        # must own the PE at startup.
        filler_at = {}
        filler_at[0] = [make_vt(0)]
        filler_at[1] = [make_vt(1)]
        filler_at[2] = [make_vt(2)]
        filler_at[3] = [make_vt(3), make_qk(1, 0)]
        filler_at[4] = [make_qk(1, 1)]
        filler_at[5] = [make_vt(4), make_qk(5, 0)]
        filler_at[6] = [make_vt(5), make_qk(5, 1)]
        filler_at[7] = [make_vt(6)]
        filler_at[8] = [make_vt(7)]
        filler_at[9] = [make_qk(2, 0)]
        filler_at[10] = [make_qk(2, 1)]
        filler_at[11] = [make_qk(6, 0)]
        filler_at[12] = [make_qk(6, 1)]
        filler_at[16] = [make_qk(3, 0)]
        filler_at[17] = [make_qk(3, 1)]
        filler_at[18] = [make_qk(7, 0)]
        filler_at[19] = [make_qk(7, 1)]

        rcp_consts = dict(s0=RECIP_APPROX_FAST_CONSTS["s0"],
                          s1=RECIP_APPROX_FAST_CONSTS["s1"],
                          imm2=RECIP_APPROX_FAST_CONSTS["imm2"])

        def div_recip(stgs, dma_eng=None):
            # Fold the four 512-wide ones-row sums into (128,16) via tiny
            # SBUF->SBUF DMAs (DMA engines are idle here) so the DVE
            # reciprocal is one cheap op, then unfold back to a partition-0
            # row for the broadcast.
            de = dma_eng if dma_eng is not None else nc.sync
            den128 = dvp.tile([128, 16], BF16, tag="d128")
            for i, (sg, e, t, mm_) in enumerate(stgs):
                de.dma_start(den128[:, 4 * i:4 * (i + 1)], sg[64:65, :])
            r128 = dvp.tile([128, 16], BF16, tag="r128")
            nc.vector._custom_dve(RECIPROCAL_APPROX_FAST, out=r128[:],
                                  in0=den128[:], **rcp_consts)
            rden = dvp.tile([1, 4 * 512], BF16, tag="rden")
            for i in range(4):
                nc.sync.dma_start(rden[0:1, 512 * i:512 * (i + 1)],
                                  r128[:, 4 * i:4 * (i + 1)])
            return rden

        def div_mul(rden, i, sg, e, t, mm_):
            bsb = dvp.tile([64, 512], BF16, tag="bsb")
            nc.gpsimd.partition_broadcast(bsb[:], rden[0:1, 512 * i:512 * (i + 1)])
            dst = a_sb[64 * e:64 * (e + 1),
                       mm_ * L + 512 * t:mm_ * L + 512 * (t + 1)]
            nc.vector.tensor_mul(dst, sg[0:64, :], bsb[:])
            if has_qkv_bias:
                nc.vector.tensor_scalar_add(
                    dst, dst, vb[64 * e:64 * (e + 1), mm_:mm_ + 1])

        def division_steps(stgs):
            # generator: one cheap step per scheduling slot
            rden = div_recip(stgs)
            yield
            for i, (sg, e, t, mm_) in enumerate(stgs):
                div_mul(rden, i, sg, e, t, mm_)
                if i % 2 == 1:
                    yield

        # ---- flat 32-block stream: block b = (m, sc). Attention matmuls lag
        # the score/exp stream by a global block count (3, tapering to 1 at
        # the end), so each pair's tail attn+staging spreads into the next
        # pair's blocks instead of lumping at pair boundaries. ----
        ps_a_of = {}
        ew_tiles = {}
        state = {"pending_div": None, "final_stgs": None, "final_rden4": None,
                 "dslice": None}
        fin_et = [(0, 0), (1, 0), (0, 1), (1, 1)]

        def q_ap(m, e, t):
            return qk[64 * e:64 * (e + 1), m * L + 512 * t:m * L + 512 * (t + 1)]

        def k_ap(m, e, sc):
            return qk[64 * e:64 * (e + 1),
                      (4 + m) * L + 128 * sc:(4 + m) * L + 128 * (sc + 1)]

        def attn_emit(m, sc):
            if m not in ps_a_of:
                pa = [[None, None], [None, None]]
                for e in range(2):
                    for t in range(TC):
                        pa[e][t] = psA.tile([65, 512], F32, tag="pa",
                                            name=f"pa_{m}_{e}_{t}")
                ps_a_of[m] = pa
            pa = ps_a_of[m]
            for e in range(2):
                ew = ew_tiles[(m, sc, e)]
                for t in range(TC):
                    nc.tensor.matmul(
                        pa[e][t][:],
                        vt[:, sc * 520 + (2 * m + e) * 65:
                           sc * 520 + (2 * m + e) * 65 + 65],
                        ew[:, 512 * t:512 * (t + 1)],
                        start=(sc == 0), stop=(sc == SC - 1))

        def stage_pair(m):
            # flush the previous pair's division before starting a new one
            if state["pending_div"] is not None:
                for _ in state["pending_div"]:
                    pass
            ps_a = ps_a_of[m]
            stgs = []
            if m == 3:
                # Final pair: no DMA fold round-trip (it would sit naked on
                # the critical path). The four denominator rows are copied
                # (split ACT/DVE) into one tile at base partitions 0/32/64
                # (cols 512:1024 of p64 for the 4th), one DVE reciprocal
                # covers all four, and the PE broadcasts from its rows during
                # proj. Value rows stage split ACT/DVE, t=0 entries first.
                den4 = dvp.tile([65, 1024], BF16, tag="den4")
                _dslice = lambda dd, i_: (dd[32 * i_:32 * i_ + 1, 0:512]
                                          if i_ < 3 else dd[64:65, 512:1024])
                for i_, (e, t) in enumerate(fin_et):
                    if i_ % 2 == 0:
                        nc.scalar.activation(_dslice(den4, i_),
                                             ps_a[e][t][64:65, :], AF.Identity)
                    else:
                        nc.vector.tensor_copy(_dslice(den4, i_),
                                              ps_a[e][t][64:65, :])
                rden4 = dvp.tile([65, 1024], BF16, tag="rden4")
                nc.vector._custom_dve(RECIPROCAL_APPROX_FAST, out=rden4[:],
                                      in0=den4[:], **rcp_consts)
                state["final_rden4"] = rden4
                state["dslice"] = _dslice
                for e, t in fin_et:
                    sg = asg.tile([64, 512], BF16, tag="astg")
                    if e == 0:
                        nc.scalar.activation(sg[:], ps_a[e][t][0:64, :],
                                             AF.Identity)
                    else:
                        nc.vector.tensor_copy(sg[:], ps_a[e][t][0:64, :])
                    stgs.append((sg, e, t, m))
                state["pending_div"] = None
                state["final_stgs"] = stgs
            else:
                for e in range(2):
                    for t in range(TC):
                        sg = asg.tile([65, 512], BF16, tag="astg")
                        nc.vector.tensor_copy(sg[:], ps_a[e][t][:])
                        stgs.append((sg, e, t, m))
                state["pending_div"] = division_steps(stgs)

        def lag_for(b):
            return 2 if b < 28 else 1

        pending_attn = []
        for b in range(4 * SC):
            m, sc = divmod(b, SC)
            ps_w = [None, None]
            for e in range(2):
                ps_w[e] = psS.tile([128, 1024], F32, tag="ps", name=f"ps_{b}_{e}")
            # packed score MM pairs (head 2m rows 0-63, head 2m+1 rows 64-127)
            for t in range(TC):
                for e in range(2):
                    nc.tensor.matmul(ps_w[e][:, 512 * t:512 * (t + 1)],
                                     k_ap(m, e, sc), q_ap(m, e, t),
                                     start=True, stop=True)
            for e in range(2):
                ew = ewp.tile([128, L], BF16, tag="ew")
                ew_tiles[(m, sc, e)] = ew
                nc.scalar.activation(ew[:], ps_w[e][:], AF.Exp)
            if debug and b == 0:
                dump_bf16(dbg["d_ew0"], ew_tiles[(0, 0, 0)][:], L)
            # pending division, one step per block to spread the load
            if state["pending_div"] is not None:
                next(state["pending_div"], None)
            # PE filler (vt chunks / deferred q,k) in the ACT-bound stream
            for f in filler_at.get(b, ()):
                f()
            if b == 16:
                # stage the residual input chunks 0-1 to bf16 for the
                # proj-tail identity matmuls (DVE has slack here)
                for ii in range(2):
                    for tt in range(TC):
                        nc.vector.tensor_copy(
                            xsb[:, ii * L + 512 * tt:ii * L + 512 * (tt + 1)],
                            xs[:, ii * L + 512 * tt:ii * L + 512 * (tt + 1)])
            # lagged attention drain
            pending_attn.append((m, sc))
            while len(pending_attn) > lag_for(b):
                am, asc = pending_attn.pop(0)
                attn_emit(am, asc)
                if asc == SC - 1:
                    stage_pair(am)
        while pending_attn:
            am, asc = pending_attn.pop(0)
            attn_emit(am, asc)
            if asc == SC - 1:
                stage_pair(am)
        final_stgs = state["final_stgs"]
        final_rden4 = state["final_rden4"]
        _dslice = state["dslice"]
        attn_psum.__exit__(None, None, None)
        attn_acc.__exit__(None, None, None)

        if debug:
            dump_bf16(dbg["d_qk"], qk[:], 8 * L)
            dump_bf16(dbg["d_vt"], vt[:], SC * HEADS * 65)
            dump_bf16(dbg["d_asb"], a_sb[:], KC * L)

        # ---- proj + residual: k-outer waves, so the first 24 matmuls only
        # need a_sb chunks 0-2 and overlap the final division flush. The
        # final pair's reciprocal broadcasts run as tiny PE outer products
        # into the (now free) psA slots; psA stays open through proj.
        # For output chunks i=0,1 the residual is folded into PSUM via an
        # identity matmul on xs_bf16 so ScalarE can produce those outputs
        # (and issue their DMAs) in parallel with DVE doing i=2,3 ----
        with tc.tile_pool(name="psP", bufs=8, space="PSUM") as psP, \
             _scope("proj"):
            # group-A output tiles preallocated first so they get PSUM slots
            # whose banks free at stream end (their k0-2 waves are the bulk
            # of the tail PE work and don't depend on the final division)
            pjt = {}
            for i, t in ((0, 0), (1, 0), (0, 1), (1, 1)):
                pjt[(i, t)] = psP.tile([128, 512], F32, tag="ps",
                                       name=f"pj{i}{t}")
            # final pair's reciprocal broadcasts (PE outer products) +
            # multiplies, t=0 entries first
            for i_, (sg, e, tt, mm_) in enumerate(final_stgs):
                pb_ps = psP.tile([64, 512], F32, tag="ps", name=f"pbc{i_}")
                bp = 32 * i_ if i_ < 3 else 64
                nc.tensor.matmul(pb_ps[:], ones64[bp:bp + 1, :],
                                 _dslice(final_rden4, i_), start=True, stop=True)
                dst = a_sb[64 * e:64 * (e + 1),
                           mm_ * L + 512 * tt:mm_ * L + 512 * (tt + 1)]
                nc.vector.tensor_mul(dst, sg[:], pb_ps[:])
                if has_qkv_bias:
                    nc.vector.tensor_scalar_add(
                        dst, dst, vb[64 * e:64 * (e + 1), mm_:mm_ + 1])
            # group A (i=0,1, residual folded in via identity matmul on
            # xs_bf16) drains through ScalarE + its DMA queue; group B
            # (i=2,3) through DVE adds + the sync queue
            for i, t in ((0, 0), (1, 0), (0, 1), (1, 1),
                         (2, 0), (3, 0), (2, 1), (3, 1)):
                ps = pjt.get((i, t))
                if ps is None:
                    ps = psP.tile([128, 512], F32, tag="ps", name=f"pj{i}{t}")
                for k in range(KC):
                    nc.tensor.matmul(
                        ps[:],
                        pw[:, k * CH + 128 * i:k * CH + 128 * (i + 1)],
                        a_sb[:, k * L + 512 * t:k * L + 512 * (t + 1)],
                        start=(k == 0), stop=(k == KC - 1 and i >= 2))
                ot = outp.tile([128, 512], F32, tag="ot")
                if i < 2:
                    nc.tensor.matmul(
                        ps[:], ident[:],
                        xsb[:, i * L + 512 * t:i * L + 512 * (t + 1)],
                        start=False, stop=True)
                    nc.scalar.activation(ot[:], ps[:], AF.Identity)
                    if has_proj_bias:
                        nc.vector.tensor_scalar_add(ot[:], ot[:], pb[:, i:i + 1])
                    nc.scalar.dma_start(
                        out_d[128 * i:128 * (i + 1), 512 * t:512 * (t + 1)], ot[:])
                else:
                    nc.vector.tensor_add(
                        ot[:], xs[:, i * L + 512 * t:i * L + 512 * (t + 1)], ps[:])
                    if has_proj_bias:
                        nc.vector.tensor_scalar_add(ot[:], ot[:], pb[:, i:i + 1])
                    nc.sync.dma_start(
                        out_d[128 * i:128 * (i + 1), 512 * t:512 * (t + 1)], ot[:])
        attn_acc.__exit__(None, None, None)
        ctx.close()

    nc.compile()
    return nc


def _prep_inputs(x, norm_w, norm_b, qkv_w, qkv_b, proj_w, proj_b):
    scale = DH ** -0.25
    w_eff = (qkv_w.astype(np.float64) * norm_w.astype(np.float64)[None, :])
    b_eff = qkv_b.astype(np.float64) + w_eff @ norm_b.astype(np.float64)
    # reference splits qkv per head: row h*192 + {0:64 q, 64:128 k, 128:192 v}.
    # device layout wants [q_all_heads | k_all_heads | v_all_heads], head-major.
    perm = np.concatenate([
        np.concatenate([np.arange(h * 3 * DH + t * DH, h * 3 * DH + (t + 1) * DH)
                        for h in range(HEADS)])
        for t in range(3)])
    w_eff = w_eff[perm]
    b_eff = b_eff[perm]
    w_eff[:2 * CH] *= scale
    b_eff[:2 * CH] *= scale
    qkv_wt = np.ascontiguousarray(w_eff.T).astype(np.float32).astype(ml_dtypes.bfloat16)
    proj_wt = np.ascontiguousarray(proj_w.T).astype(ml_dtypes.bfloat16)

    p = np.arange(128)
    gmask = (p[:, None] // 16 == np.arange(8)[None, :]).astype(np.float32)
    gmask_t = np.ascontiguousarray(gmask.T)

    has_qkv_bias = bool(np.any(b_eff != 0.0))
    has_proj_bias = bool(np.any(proj_b != 0.0))
    common = {"qkv_wt": qkv_wt, "proj_wt": proj_wt, "gmask": gmask,
              "gmask_t": gmask_t,
              "ident": np.eye(128, dtype=np.float32).astype(ml_dtypes.bfloat16)}
    if has_qkv_bias:
        qk_part = b_eff[:2 * CH].astype(np.float32).reshape(8, 128).T
        v_part = b_eff[2 * CH:].astype(np.float32).reshape(KC, 128).T
        common["qk_bias"] = np.ascontiguousarray(qk_part)
        common["v_bias"] = np.ascontiguousarray(v_part)
    if has_proj_bias:
        common["p_bias"] = np.ascontiguousarray(
            proj_b.astype(np.float32).reshape(KC, 128).T)
    xf = np.ascontiguousarray(x.reshape(B, CH, L)).astype(np.float32)
    in_maps = [dict(common, x=np.ascontiguousarray(xf[i])) for i in range(B)]
    return in_maps, has_qkv_bias, has_proj_bias


def _get_nc(flags):
    if flags not in _cache:
        _cache[flags] = _build(*flags)
    return _cache[flags]


def _run(inputs, trace=False, tmpdir=None):
    import time
    from concourse.bass_utils import run_bass_kernel_spmd
    in_maps, hqb, hpb = _prep_inputs(**inputs)
    nc = _get_nc((hqb, hpb))
    kw = {}
    if trace:
        kw = dict(trace=True, tmpdir=tmpdir)
    last_err = None
    for attempt in range(3):
        # the very first execution on a freshly-attached device occasionally
        # fails with NRT_EXEC_UNIT_UNRECOVERABLE; a retry recovers it
        try:
            res = run_bass_kernel_spmd(nc, in_maps, list(range(B)), **kw)
            break
        except Exception as e:  # noqa: BLE001
            last_err = e
            time.sleep(5)
    else:
        raise last_err
    out = np.stack([res.results[i]["out"] for i in range(B)])
    return out.reshape(B, CH, HH, WW).astype(np.float32), res


def kernel(x, norm_w, norm_b, qkv_w, qkv_b, proj_w, proj_b):
    out, _ = _run(dict(x=x, norm_w=norm_w, norm_b=norm_b, qkv_w=qkv_w,
                       qkv_b=qkv_b, proj_w=proj_w, proj_b=proj_b))
    return out
